# revision 27
# baseline (speedup 1.0000x reference)
"""Trainium2 Bass kernel for nn_ConvAttention (N=8, C=512, L=2048, 8 heads, causal).

Sharding: data-parallel over the batch dim N=8 -> one batch per NeuronCore.

v2 schedule: the ACT engine (softmax exp, 1 elem/cycle/lane) is the pacemaker;
the PE is kept continuously busy (to hold the 2.4 GHz p-state) by deferring the
K/V/Q/Wo projections into the attention loop as fine-grained filler matmuls.
Host passes x and the four weights pre-transposed in bf16 (no on-chip W
transpose or cast). The softmax reciprocal runs on DVE (reciprocal_approx_fast)
so the ACT engine only ever uses the EXP table (no table reloads), and the
denominator partition-broadcast matmuls run in fp32r (1 cycle/row, not 4).
"""

import numpy as np
from contextlib import ExitStack

try:
    import concourse.bass as bass
except ImportError:  # concourse is on PYTHONPATH in the target container
    import sys
    sys.path.insert(0, "/opt/trn_rl_repo")
    import concourse.bass as bass

import concourse.tile as tile
from concourse import bacc, mybir
from concourse.bass_utils import run_bass_kernel_spmd

F32 = mybir.dt.float32
F32R = mybir.dt.float32r
BF16 = mybir.dt.bfloat16
EXP = mybir.ActivationFunctionType.Exp
LN = mybir.ActivationFunctionType.Ln

N_CORES = 8
N, C, L = 8, 512, 2048
H = 8
D = C // H            # 64
P = 128
CT = C // P           # 4 channel tiles
QBLK = 512            # q tile (matmul free dim)
NQT = L // QBLK       # 4 q tiles
HP = H // 2           # 4 head pairs (one per 128-channel tile)
SCALE = float(C) ** -0.5

W_NAMES = ("wq", "wk", "wv", "wo")


def _emit(nc):
    # host passes x in bf16 and each W pre-transposed (wT[c, o] = W[o, c]) in
    # bf16, so SBUF tiles load straight off DMA.
    x_d = nc.dram_tensor("x", [C, L], BF16, kind="ExternalInput").ap()
    wt_d = {nm: nc.dram_tensor(nm + "t", [C, C], BF16, kind="ExternalInput").ap()
            for nm in W_NAMES}
    bo_d = nc.dram_tensor("bo", [C], F32, kind="ExternalInput").ap()
    y_d = nc.dram_tensor("y", [C, L], F32, kind="ExternalOutput").ap()
    y_r = y_d.rearrange("(t p) l -> t p l", p=P)

    with tile.TileContext(nc) as tc, ExitStack() as ctx:
        const = ctx.enter_context(tc.tile_pool(name="const", bufs=1))
        persist = ctx.enter_context(tc.tile_pool(name="persist", bufs=1))

        bo_sb = const.tile([P, CT], F32, tag="bo", name="bo_sb")
        nc.sync.dma_start(bo_sb, bo_d.rearrange("(t p) -> p t", p=P))
        onesH = const.tile([P, H], F32, tag="onesH", name="onesH")
        nc.vector.memset(onesH, 1.0)

        # ---- persistent SBUF tensors
        wT = {nm: [persist.tile([P, C], BF16, tag=f"{nm}T{ct}", name=f"{nm}T{ct}")
                   for ct in range(CT)] for nm in W_NAMES}
        x_sb = [persist.tile([P, L], BF16, tag=f"x{ct}", name=f"x{ct}")
                for ct in range(CT)]
        k_sb = [persist.tile([P, L], BF16, tag=f"k{ot}", name=f"k{ot}")
                for ot in range(CT)]
        vt_sb = [persist.tile([P, H, D + 1], BF16, tag=f"vt{lt}", name=f"vt{lt}")
                 for lt in range(L // P)]

        # DMA order: wk first (K projection starts earliest), then x, wq, wv;
        # wo is only needed once qt0 finishes.
        wt_r = {nm: wt_d[nm].rearrange("(t p) o -> t p o", p=P) for nm in W_NAMES}
        x_r = x_d.rearrange("(t p) l -> t p l", p=P)
        for ct in range(CT):
            nc.sync.dma_start(wT["wk"][ct], wt_r["wk"][ct])
        # x arrives in 512-column chunks so the first projections (which only
        # need columns 0-511) start ~4x earlier
        for ct in range(CT):
            nc.sync.dma_start(x_sb[ct][:, 0:QBLK], x_r[ct][:, 0:QBLK])
        for ct in range(CT):
            nc.sync.dma_start(wT["wq"][ct], wt_r["wq"][ct])
        for ct in range(CT):
            nc.sync.dma_start(wT["wv"][ct], wt_r["wv"][ct])
        for lc in range(1, 4):
            for ct in range(CT):
                nc.sync.dma_start(x_sb[ct][:, lc * QBLK:(lc + 1) * QBLK],
                                  x_r[ct][:, lc * QBLK:(lc + 1) * QBLK])
        for ct in range(CT):
            nc.sync.dma_start(wT["wo"][ct], wt_r["wo"][ct])

        ps_proj = ctx.enter_context(tc.tile_pool(name="ps_proj", bufs=2, space="PSUM"))
        q_pool = ctx.enter_context(tc.tile_pool(name="q", bufs=2))
        oc_pool = ctx.enter_context(tc.tile_pool(name="oc", bufs=2))
        pt_pool = ctx.enter_context(tc.tile_pool(name="pt", bufs=4))
        nrm_pool = ctx.enter_context(tc.tile_pool(name="nrm", bufs=2))
        y_pool = ctx.enter_context(tc.tile_pool(name="y", bufs=2))
        ps_st = ctx.enter_context(tc.tile_pool(name="ps_st", bufs=2, space="PSUM"))
        ps_av = ctx.enter_context(tc.tile_pool(name="ps_av", bufs=2, space="PSUM"))

        # ---- projection helpers: each returns a list of unit thunks (one
        # matmul each + a final DVE staging copy) so filler work drips into
        # the attention loop at single-matmul (~213ns) granularity.
        def proj_units(lhsT_of, rhs_of, fin_fn):
            def run():
                ps = ps_proj.tile([P, QBLK], F32, tag="proj", name="proj_ps")
                for ct in range(CT):
                    nc.tensor.matmul(
                        ps, lhsT=lhsT_of(ct), rhs=rhs_of(ct),
                        start=(ct == 0), stop=(ct == CT - 1))
                fin_fn(ps)
            return [run]

        def k_units(ot, lc):
            return proj_units(
                lambda ct: wT["wk"][ct][:, ot * P:(ot + 1) * P],
                lambda ct: x_sb[ct][:, lc * QBLK:(lc + 1) * QBLK],
                lambda ps: nc.vector.tensor_copy(
                    k_sb[ot][:, lc * QBLK:(lc + 1) * QBLK], ps))

        def v_units(lt):
            def fin(ps):
                t = vt_sb[lt]
                nc.vector.tensor_copy(t[:, :, D], onesH)
                nc.vector.tensor_copy(
                    t[:, :, 0:D], ps.rearrange("p (h d) -> p h d", d=D))
            return proj_units(
                lambda ct: x_sb[ct][:, lt * P:(lt + 1) * P],
                lambda ct: wT["wv"][ct], fin)

        q_tiles = {}

        def q_units(qt, ot):
            def fin(ps):
                nc.vector.tensor_copy(q_tiles[qt][:, ot, :], ps)
            units = proj_units(
                lambda ct: wT["wq"][ct][:, ot * P:(ot + 1) * P],
                lambda ct: x_sb[ct][:, qt * QBLK:(qt + 1) * QBLK], fin)
            first = units[0]

            def f0():
                if qt not in q_tiles:
                    q_tiles[qt] = q_pool.tile([P, CT, QBLK], BF16, tag="q",
                                              name="q_sb")
                first()
            units[0] = f0
            return units

        oc_tiles = {}

        def wo_units(qt, ot):
            def fin(ps):
                ysb = y_pool.tile([P, QBLK], F32, tag="y", name="y_sb")
                nc.vector.tensor_tensor(
                    ysb, ps, bo_sb[:, ot:ot + 1].to_broadcast((P, QBLK)),
                    mybir.AluOpType.add)
                nc.sync.dma_start(y_r[ot][:, qt * QBLK:(qt + 1) * QBLK], ysb)
            return proj_units(
                lambda ct: wT["wo"][ct][:, ot * P:(ot + 1) * P],
                lambda ct: oc_tiles[qt][ct], fin)

        def run_units(units):
            for u in units:
                u()

        # ---- warmup: the minimum for (qt0, hp0) to start
        run_units(k_units(0, 0))
        run_units(q_units(0, 0))
        run_units(v_units(0))

        # ---- filler queue: remaining projection work in need-order, drained
        # into the attention loop as PE filler. `need(id)` force-drains the
        # queue through a required producer; a steady drip keeps the PE fed.
        unit_q = []
        done = {("k", 0, 0), ("q", 0, 0), ("v", 0)}

        def enq(fid, units):
            for u in units[:-1]:
                unit_q.append((None, u))
            unit_q.append((fid, units[-1]))

        for lt in (1, 2, 3):
            enq(("v", lt), v_units(lt))
        for ot in (1, 2, 3):
            enq(("k", ot, 0), k_units(ot, 0))
            enq(("q", 0, ot), q_units(0, ot))
        for qt in (1, 2, 3):
            enq(("k", 0, qt), k_units(0, qt))
            enq(("q", qt, 0), q_units(qt, 0))
            for lt in range(4 * qt, 4 * qt + 4):
                enq(("v", lt), v_units(lt))
            for ot in (1, 2, 3):
                enq(("k", ot, qt), k_units(ot, qt))
                enq(("q", qt, ot), q_units(qt, ot))

        # Wo(qt-1) is reserved for qt's last head-pair so the PE stays at full
        # clock right up to the output tail
        wo_qs = {qt: [u for ot in range(CT) for u in wo_units(qt - 1, ot)]
                 for qt in (1, 2, 3)}

        def need(fid):
            if fid in done:
                return
            while unit_q:
                i, fn = unit_q.pop(0)
                fn()
                if i is not None:
                    done.add(i)
                    if i == fid:
                        return

        FILL_PER_KT = 0.4  # closures per kt (~340ns of PE work per kt slot)
        fill_acc = [0.0]

        def drip(qt, hp, kt, nkt):
            wq = wo_qs.get(qt) if hp == 3 else None
            if wq and kt % max(nkt // 4, 1) == 1:
                wq.pop(0)()
                return
            fill_acc[0] += FILL_PER_KT
            if unit_q and fill_acc[0] >= 1.0:
                fill_acc[0] -= 1.0
                i, fn = unit_q.pop(0)
                fn()
                if i is not None:
                    done.add(i)

        # ---- attention
        pend_norm = [None]
        wo3_part = {}

        def run_pend_norm():
            if pend_norm[0] is not None:
                pend_norm[0]()
                pend_norm[0] = None

        for qt in range(NQT):
            oc_tiles[qt] = [oc_pool.tile([P, QBLK], BF16, tag=f"oc{j}",
                                         name=f"oc{j}") for j in range(CT)]
            oc = oc_tiles[qt]

            for hp in range(HP):
                need(("k", hp, qt))
                need(("q", qt, hp))
                q_sb = q_tiles[qt]
                nkt = 4 * qt + 4
                av = [ps_av.tile([65, QBLK], F32, tag="av", name="av_ps")
                      for _ in range(2)]
                prev = None  # (pt, kt, co, cols)
                for kt in range(nkt):
                    j = kt - 4 * qt          # >=0 -> diagonal block index
                    co = 0 if j < 0 else P * j
                    cols = QBLK - co
                    # head a's S^T in PSUM bank 0, head b's in bank 1 (two
                    # concurrent row-group matmuls must not share a bank)
                    stp = ps_st.tile([P, 2 * QBLK], F32, tag="st", name="st_ps")
                    for sub, ofs in ((0, 0), (1, QBLK)):
                        pofs = sub * D
                        nc.tensor.matmul(
                            stp[:, ofs:ofs + cols],
                            lhsT=k_sb[hp][pofs:pofs + D, kt * P:(kt + 1) * P],
                            rhs=q_sb[pofs:pofs + D, hp, co:QBLK],
                            start=True, stop=True)
                    pt = pt_pool.tile([P, 2 * QBLK], BF16, tag="pt", name="pt_sb")
                    sv = stp.rearrange("p (g c) -> p g c", c=QBLK)[:, :, 0:cols]
                    pv = pt.rearrange("p (g c) -> p g c", c=QBLK)[:, :, 0:cols]
                    nc.scalar.activation(pv, sv, EXP, scale=SCALE)
                    if j >= 0:
                        for ofs in (0, QBLK):
                            sl = pt[:, ofs:ofs + cols]
                            nc.gpsimd.affine_select(
                                out=sl, in_=sl,
                                compare_op=mybir.AluOpType.is_ge, fill=0.0,
                                base=0, channel_multiplier=-1,
                                pattern=[[1, cols]])
                    if kt == 1:
                        run_pend_norm()
                    drip(qt, hp, kt, nkt)
                    # pre-run Wo(qt3) ct0-2 for two output blocks during the
                    # final head-pair (held-open PSUM groups): only 2+8
                    # matmuls remain after the last norm
                    if qt == 3 and hp == 3 and kt in (nkt - 2, nkt - 1):
                        pot = kt - (nkt - 2)
                        wps = ps_proj.tile([P, QBLK], F32, tag="proj",
                                           name="proj_ps")
                        for ct in range(3):
                            nc.tensor.matmul(
                                wps,
                                lhsT=wT["wo"][ct][:, pot * P:(pot + 1) * P],
                                rhs=oc[ct], start=(ct == 0), stop=False,
                                skip_group_check=True)
                        wo3_part[pot] = wps
                    if prev is not None:
                        ppt, pkt, pco, pcols = prev
                        need(("v", pkt))
                        for sub, ofs in ((0, 0), (1, QBLK)):
                            nc.tensor.matmul(
                                av[sub][:, pco:QBLK],
                                lhsT=vt_sb[pkt][:, 2 * hp + sub, :],
                                rhs=ppt[:, ofs:ofs + pcols],
                                start=(pkt == 0), stop=True,
                                skip_group_check=True)
                    prev = (pt, kt, co, cols)
                ppt, pkt, pco, pcols = prev
                need(("v", pkt))
                for sub, ofs in ((0, 0), (1, QBLK)):
                    nc.tensor.matmul(
                        av[sub][:, pco:QBLK],
                        lhsT=vt_sb[pkt][:, 2 * hp + sub, :],
                        rhs=ppt[:, ofs:ofs + pcols],
                        start=(pkt == 0), stop=True,
                        skip_group_check=True)

                # denominator rows to a partition-0 tile (partition_broadcast
                # reads the tile's physical partition 0), reciprocal on DVE,
                # then stage the AV values to SBUF (frees the accumulators)
                den0 = nrm_pool.tile([1, 2, QBLK], F32, tag="den0", name="den0")
                for sub in range(2):
                    nc.vector.tensor_copy(den0[:, sub, :], av[sub][64:65, :])
                nc.vector.reciprocal_approx_fast(den0, den0)
                avs = nrm_pool.tile([D, 2, QBLK], F32, tag="avs", name="avs")
                for sub in range(2):
                    nc.vector.tensor_copy(avs[:, sub, :], av[sub][0:D, :])

                def norm_tail(hp=hp, avs=avs, den0=den0, oc=oc):
                    for sub in range(2):
                        bc = nrm_pool.tile([D, QBLK], F32, tag="bc", name="bc")
                        nc.gpsimd.partition_broadcast(bc, den0[0:1, sub, :])
                        if sub == 0:
                            nc.vector.tensor_mul(
                                oc[hp][0:D, :], avs[:, sub, :], bc)
                        else:
                            tmp = nrm_pool.tile([D, QBLK], BF16, tag="tmp",
                                                name="tmp")
                            nc.vector.tensor_mul(
                                tmp, avs[:, sub, :], bc)
                            nc.sync.dma_start(oc[hp][D:P, :], tmp)
                pend_norm[0] = norm_tail

            run_pend_norm()
            for u in wo_qs.pop(qt, []):
                u()

        # tail: drain leftover fillers and the last Wo projection
        while unit_q:
            i, fn = unit_q.pop(0)
            fn()
        for ot in range(CT):
            if ot in wo3_part:
                wps = wo3_part[ot]
                nc.tensor.matmul(
                    wps, lhsT=wT["wo"][3][:, ot * P:(ot + 1) * P],
                    rhs=oc_tiles[3][3], start=False, stop=True,
                    skip_group_check=True)
                ysb = y_pool.tile([P, QBLK], F32, tag="y", name="y_sb")
                nc.vector.tensor_tensor(
                    ysb, wps, bo_sb[:, ot:ot + 1].to_broadcast((P, QBLK)),
                    mybir.AluOpType.add)
                nc.sync.dma_start(y_r[ot][:, 3 * QBLK:4 * QBLK], ysb)
            else:
                run_units(wo_units(3, ot))


_CACHE = {}


def _get_program():
    if "nc" not in _CACHE:
        nc = bacc.Bacc("TRN2", target_bir_lowering=False, debug=False,
                       num_devices=N_CORES)
        _emit(nc)
        nc.compile()
        _CACHE["nc"] = nc
    return _CACHE["nc"]


def _run(inputs, trace=False, **kwargs):
    import ml_dtypes
    nc = _get_program()
    bf16 = ml_dtypes.bfloat16
    x = np.ascontiguousarray(np.asarray(inputs["x"], dtype=np.float32)).astype(bf16)
    shared = {nm + "t": np.ascontiguousarray(
                  np.asarray(inputs[nm], dtype=np.float32).T).astype(bf16)
              for nm in W_NAMES}
    shared["bo"] = np.ascontiguousarray(np.asarray(inputs["bo"], dtype=np.float32))
    in_maps = [{"x": np.ascontiguousarray(x[i]), **shared} for i in range(N_CORES)]
    res = run_bass_kernel_spmd(nc, in_maps, core_ids=list(range(N_CORES)),
                               trace=trace, **kwargs)
    y = np.stack([np.asarray(res.results[i]["y"]) for i in range(N_CORES)], axis=0)
    return y, res


def kernel(x, Wq, Wk, Wv, Wo, bo):
    y, _ = _run({"x": x, "wq": Wq, "wk": Wk, "wv": Wv, "wo": Wo, "bo": bo})
    return y


# revision 30
# speedup vs baseline: 1.0526x; 1.0526x over previous
"""Trainium2 Bass kernel for nn_ConvAttention (N=8, C=512, L=2048, 8 heads, causal).

Sharding: data-parallel over the batch dim N=8 -> one batch per NeuronCore.

v2 schedule: the ACT engine (softmax exp, 1 elem/cycle/lane) is the pacemaker;
the PE is kept continuously busy (to hold the 2.4 GHz p-state) by deferring the
K/V/Q/Wo projections into the attention loop as fine-grained filler matmuls.
Host passes x and the four weights pre-transposed in bf16 (no on-chip W
transpose or cast). The softmax reciprocal runs on DVE (reciprocal_approx_fast)
so the ACT engine only ever uses the EXP table (no table reloads), and the
denominator partition-broadcast matmuls run in fp32r (1 cycle/row, not 4).
"""

import numpy as np
from contextlib import ExitStack

try:
    import concourse.bass as bass
except ImportError:  # concourse is on PYTHONPATH in the target container
    import sys
    sys.path.insert(0, "/opt/trn_rl_repo")
    import concourse.bass as bass

import concourse.tile as tile
from concourse import bacc, mybir
from concourse.bass_utils import run_bass_kernel_spmd

F32 = mybir.dt.float32
F32R = mybir.dt.float32r
BF16 = mybir.dt.bfloat16
EXP = mybir.ActivationFunctionType.Exp
LN = mybir.ActivationFunctionType.Ln

N_CORES = 8
N, C, L = 8, 512, 2048
H = 8
D = C // H            # 64
P = 128
CT = C // P           # 4 channel tiles
QBLK = 512            # q tile (matmul free dim)
NQT = L // QBLK       # 4 q tiles
HP = H // 2           # 4 head pairs (one per 128-channel tile)
SCALE = float(C) ** -0.5

W_NAMES = ("wq", "wk", "wv", "wo")


def _emit(nc):
    # host passes x in bf16 and each W pre-transposed (wT[c, o] = W[o, c]) in
    # bf16, so SBUF tiles load straight off DMA.
    x_d = nc.dram_tensor("x", [C, L], BF16, kind="ExternalInput").ap()
    wt_d = {nm: nc.dram_tensor(nm + "t", [C, C], BF16, kind="ExternalInput").ap()
            for nm in W_NAMES}
    bo_d = nc.dram_tensor("bo", [C], F32, kind="ExternalInput").ap()
    y_d = nc.dram_tensor("y", [C, L], F32, kind="ExternalOutput").ap()
    y_r = y_d.rearrange("(t p) l -> t p l", p=P)

    with tile.TileContext(nc) as tc, ExitStack() as ctx:
        const = ctx.enter_context(tc.tile_pool(name="const", bufs=1))
        persist = ctx.enter_context(tc.tile_pool(name="persist", bufs=1))

        bo_sb = const.tile([P, CT], F32, tag="bo", name="bo_sb")
        nc.sync.dma_start(bo_sb, bo_d.rearrange("(t p) -> p t", p=P))
        onesH = const.tile([P, H], F32, tag="onesH", name="onesH")
        nc.vector.memset(onesH, 1.0)

        # ---- persistent SBUF tensors
        wT = {nm: [persist.tile([P, C], BF16, tag=f"{nm}T{ct}", name=f"{nm}T{ct}")
                   for ct in range(CT)] for nm in W_NAMES}
        x_sb = [persist.tile([P, L], BF16, tag=f"x{ct}", name=f"x{ct}")
                for ct in range(CT)]
        k_sb = [persist.tile([P, L], BF16, tag=f"k{ot}", name=f"k{ot}")
                for ot in range(CT)]
        vt_sb = [persist.tile([P, H, D + 1], BF16, tag=f"vt{lt}", name=f"vt{lt}")
                 for lt in range(L // P)]

        # DMA order: wk first (K projection starts earliest), then x, wq, wv;
        # wo is only needed once qt0 finishes.
        wt_r = {nm: wt_d[nm].rearrange("(t p) o -> t p o", p=P) for nm in W_NAMES}
        x_r = x_d.rearrange("(t p) l -> t p l", p=P)
        for ct in range(CT):
            nc.sync.dma_start(wT["wk"][ct], wt_r["wk"][ct])
        # x arrives in 512-column chunks so the first projections (which only
        # need columns 0-511) start ~4x earlier
        for ct in range(CT):
            nc.sync.dma_start(x_sb[ct][:, 0:QBLK], x_r[ct][:, 0:QBLK])
        for ct in range(CT):
            nc.sync.dma_start(wT["wq"][ct], wt_r["wq"][ct])
        for ct in range(CT):
            nc.sync.dma_start(wT["wv"][ct], wt_r["wv"][ct])
        for lc in range(1, 4):
            for ct in range(CT):
                nc.sync.dma_start(x_sb[ct][:, lc * QBLK:(lc + 1) * QBLK],
                                  x_r[ct][:, lc * QBLK:(lc + 1) * QBLK])
        for ct in range(CT):
            nc.sync.dma_start(wT["wo"][ct], wt_r["wo"][ct])

        ps_proj = ctx.enter_context(tc.tile_pool(name="ps_proj", bufs=2, space="PSUM"))
        q_pool = ctx.enter_context(tc.tile_pool(name="q", bufs=2))
        oc_pool = ctx.enter_context(tc.tile_pool(name="oc", bufs=2))
        pt_pool = ctx.enter_context(tc.tile_pool(name="pt", bufs=4))
        nrm_pool = ctx.enter_context(tc.tile_pool(name="nrm", bufs=2))
        y_pool = ctx.enter_context(tc.tile_pool(name="y", bufs=2))
        ps_st = ctx.enter_context(tc.tile_pool(name="ps_st", bufs=2, space="PSUM"))
        ps_av = ctx.enter_context(tc.tile_pool(name="ps_av", bufs=2, space="PSUM"))

        # ---- projection helpers: each returns a list of unit thunks (one
        # matmul each + a final DVE staging copy) so filler work drips into
        # the attention loop at single-matmul (~213ns) granularity.
        def proj_units(lhsT_of, rhs_of, fin_fn):
            def run():
                ps = ps_proj.tile([P, QBLK], F32, tag="proj", name="proj_ps")
                for ct in range(CT):
                    nc.tensor.matmul(
                        ps, lhsT=lhsT_of(ct), rhs=rhs_of(ct),
                        start=(ct == 0), stop=(ct == CT - 1))
                fin_fn(ps)
            return [run]

        def k_units(ot, lc):
            return proj_units(
                lambda ct: wT["wk"][ct][:, ot * P:(ot + 1) * P],
                lambda ct: x_sb[ct][:, lc * QBLK:(lc + 1) * QBLK],
                lambda ps: nc.vector.tensor_copy(
                    k_sb[ot][:, lc * QBLK:(lc + 1) * QBLK], ps))

        def v_units(lt):
            def fin(ps):
                t = vt_sb[lt]
                nc.vector.tensor_copy(t[:, :, D], onesH)
                nc.vector.tensor_copy(
                    t[:, :, 0:D], ps.rearrange("p (h d) -> p h d", d=D))
            return proj_units(
                lambda ct: x_sb[ct][:, lt * P:(lt + 1) * P],
                lambda ct: wT["wv"][ct], fin)

        q_tiles = {}

        def q_units(qt, ot):
            def fin(ps):
                nc.vector.tensor_copy(q_tiles[qt][:, ot, :], ps)
            units = proj_units(
                lambda ct: wT["wq"][ct][:, ot * P:(ot + 1) * P],
                lambda ct: x_sb[ct][:, qt * QBLK:(qt + 1) * QBLK], fin)
            first = units[0]

            def f0():
                if qt not in q_tiles:
                    q_tiles[qt] = q_pool.tile([P, CT, QBLK], BF16, tag="q",
                                              name="q_sb")
                first()
            units[0] = f0
            return units

        oc_tiles = {}

        def wo_units(qt, ot):
            def fin(ps):
                ysb = y_pool.tile([P, QBLK], F32, tag="y", name="y_sb")
                nc.vector.tensor_tensor(
                    ysb, ps, bo_sb[:, ot:ot + 1].to_broadcast((P, QBLK)),
                    mybir.AluOpType.add)
                nc.sync.dma_start(y_r[ot][:, qt * QBLK:(qt + 1) * QBLK], ysb)
            return proj_units(
                lambda ct: wT["wo"][ct][:, ot * P:(ot + 1) * P],
                lambda ct: oc_tiles[qt][ct], fin)

        def run_units(units):
            for u in units:
                u()

        # ---- warmup: the minimum for (qt0, hp0) to start
        run_units(k_units(0, 0))
        run_units(q_units(0, 0))
        run_units(v_units(0))

        # ---- filler queue: remaining projection work in need-order, drained
        # into the attention loop as PE filler. `need(id)` force-drains the
        # queue through a required producer; a steady drip keeps the PE fed.
        unit_q = []
        done = {("k", 0, 0), ("q", 0, 0), ("v", 0)}

        def enq(fid, units):
            for u in units[:-1]:
                unit_q.append((None, u))
            unit_q.append((fid, units[-1]))

        for lt in (1, 2, 3):
            enq(("v", lt), v_units(lt))
        for ot in (1, 2, 3):
            enq(("k", ot, 0), k_units(ot, 0))
            enq(("q", 0, ot), q_units(0, ot))
        for qt in (1, 2, 3):
            enq(("k", 0, qt), k_units(0, qt))
            enq(("q", qt, 0), q_units(qt, 0))
            for lt in range(4 * qt, 4 * qt + 4):
                enq(("v", lt), v_units(lt))
            for ot in (1, 2, 3):
                enq(("k", ot, qt), k_units(ot, qt))
                enq(("q", qt, ot), q_units(qt, ot))

        # Wo(qt-1) is reserved for qt's last head-pair so the PE stays at full
        # clock right up to the output tail
        wo_qs = {qt: [u for ot in range(CT) for u in wo_units(qt - 1, ot)]
                 for qt in (1, 2, 3)}

        def need(fid):
            if fid in done:
                return
            while unit_q:
                i, fn = unit_q.pop(0)
                fn()
                if i is not None:
                    done.add(i)
                    if i == fid:
                        return

        FILL_PER_KT = 0.4  # closures per kt (~340ns of PE work per kt slot)
        fill_acc = [0.0]

        def drip(qt, hp, kt, nkt):
            wq = wo_qs.get(qt) if hp == 3 else None
            if wq and kt % max(nkt // 4, 1) == 1:
                wq.pop(0)()
                return
            fill_acc[0] += FILL_PER_KT
            if unit_q and fill_acc[0] >= 1.0:
                fill_acc[0] -= 1.0
                i, fn = unit_q.pop(0)
                fn()
                if i is not None:
                    done.add(i)

        # ---- attention
        pend_norm = [None]
        wo3_part = {}

        def run_pend_norm():
            if pend_norm[0] is not None:
                pend_norm[0]()
                pend_norm[0] = None

        for qt in range(NQT):
            oc_tiles[qt] = [oc_pool.tile([P, QBLK], BF16, tag=f"oc{j}",
                                         name=f"oc{j}") for j in range(CT)]
            oc = oc_tiles[qt]

            for hp in range(HP):
                need(("k", hp, qt))
                need(("q", qt, hp))
                q_sb = q_tiles[qt]
                nkt = 4 * qt + 4
                av = [ps_av.tile([65, QBLK], F32, tag="av", name="av_ps")
                      for _ in range(2)]
                prev = None  # (pt, kt, co, cols)
                for kt in range(nkt):
                    j = kt - 4 * qt          # >=0 -> diagonal block index
                    co = 0 if j < 0 else P * j
                    cols = QBLK - co
                    # head a's S^T in PSUM bank 0, head b's in bank 1 (two
                    # concurrent row-group matmuls must not share a bank)
                    stp = ps_st.tile([P, 2 * QBLK], F32, tag="st", name="st_ps")
                    for sub, ofs in ((0, 0), (1, QBLK)):
                        pofs = sub * D
                        nc.tensor.matmul(
                            stp[:, ofs:ofs + cols],
                            lhsT=k_sb[hp][pofs:pofs + D, kt * P:(kt + 1) * P],
                            rhs=q_sb[pofs:pofs + D, hp, co:QBLK],
                            start=True, stop=True)
                    pt = pt_pool.tile([P, 2 * QBLK], BF16, tag="pt", name="pt_sb")
                    sv = stp.rearrange("p (g c) -> p g c", c=QBLK)[:, :, 0:cols]
                    pv = pt.rearrange("p (g c) -> p g c", c=QBLK)[:, :, 0:cols]
                    nc.scalar.activation(pv, sv, EXP, scale=SCALE)
                    if j >= 0:
                        for ofs in (0, QBLK):
                            sl = pt[:, ofs:ofs + cols]
                            nc.gpsimd.affine_select(
                                out=sl, in_=sl,
                                compare_op=mybir.AluOpType.is_ge, fill=0.0,
                                base=0, channel_multiplier=-1,
                                pattern=[[1, cols]])
                    if kt == 1:
                        run_pend_norm()
                    drip(qt, hp, kt, nkt)
                    # pre-run Wo(qt3) ct0-2 for two output blocks during the
                    # final head-pair (held-open PSUM groups): only 2+8
                    # matmuls remain after the last norm
                    if qt == 3 and hp == 3 and kt in (nkt - 2, nkt - 1):
                        pot = kt - (nkt - 2)
                        wps = ps_proj.tile([P, QBLK], F32, tag="proj",
                                           name="proj_ps")
                        for ct in range(3):
                            nc.tensor.matmul(
                                wps,
                                lhsT=wT["wo"][ct][:, pot * P:(pot + 1) * P],
                                rhs=oc[ct], start=(ct == 0), stop=False,
                                skip_group_check=True)
                        wo3_part[pot] = wps
                    if prev is not None:
                        ppt, pkt, pco, pcols = prev
                        need(("v", pkt))
                        for sub, ofs in ((0, 0), (1, QBLK)):
                            nc.tensor.matmul(
                                av[sub][:, pco:QBLK],
                                lhsT=vt_sb[pkt][:, 2 * hp + sub, :],
                                rhs=ppt[:, ofs:ofs + pcols],
                                start=(pkt == 0), stop=True,
                                skip_group_check=True)
                    prev = (pt, kt, co, cols)
                ppt, pkt, pco, pcols = prev
                need(("v", pkt))
                for sub, ofs in ((0, 0), (1, QBLK)):
                    nc.tensor.matmul(
                        av[sub][:, pco:QBLK],
                        lhsT=vt_sb[pkt][:, 2 * hp + sub, :],
                        rhs=ppt[:, ofs:ofs + pcols],
                        start=(pkt == 0), stop=True,
                        skip_group_check=True)

                # stage AV to SBUF first (frees the PSUM accumulators for the
                # next head-pair ASAP), then denominator row to a partition-0
                # tile (partition_broadcast reads physical partition 0) and
                # reciprocal on DVE
                avs = nrm_pool.tile([65, 2, QBLK], F32, tag="avs", name="avs")
                for sub in range(2):
                    nc.vector.tensor_copy(avs[:, sub, :], av[sub])
                den0 = nrm_pool.tile([1, 2, QBLK], F32, tag="den0", name="den0")
                nc.vector.tensor_copy(den0, avs[64:65, :, :])
                nc.vector.reciprocal_approx_fast(den0, den0)

                def norm_tail(hp=hp, avs=avs, den0=den0, oc=oc):
                    bc = nrm_pool.tile([D, 2, QBLK], F32, tag="bc", name="bc")
                    nc.gpsimd.partition_broadcast(bc, den0)
                    nc.vector.tensor_mul(
                        oc[hp][0:D, :], avs[0:D, 0, :], bc[:, 0, :])
                    tmp = nrm_pool.tile([D, QBLK], BF16, tag="tmp", name="tmp")
                    nc.vector.tensor_mul(tmp, avs[0:D, 1, :], bc[:, 1, :])
                    nc.sync.dma_start(oc[hp][D:P, :], tmp)
                pend_norm[0] = norm_tail

            run_pend_norm()
            if qt == NQT - 1:
                # dummy matmuls keep the PE p-state high through the final
                # norm chain so the tail Wo matmuls run at full clock
                for _ in range(16):
                    dps = ps_st.tile([P, 2 * QBLK], F32, tag="st", name="st_ps")
                    nc.tensor.matmul(
                        dps[:, 0:QBLK], lhsT=wT["wo"][0][:, 0:P],
                        rhs=x_sb[0][:, 0:QBLK], start=True, stop=True)
            for u in wo_qs.pop(qt, []):
                u()

        # tail: drain leftover fillers and the last Wo projection
        while unit_q:
            i, fn = unit_q.pop(0)
            fn()
        for ot in range(CT):
            if ot in wo3_part:
                wps = wo3_part[ot]
                nc.tensor.matmul(
                    wps, lhsT=wT["wo"][3][:, ot * P:(ot + 1) * P],
                    rhs=oc_tiles[3][3], start=False, stop=True,
                    skip_group_check=True)
                ysb = y_pool.tile([P, QBLK], F32, tag="y", name="y_sb")
                nc.vector.tensor_tensor(
                    ysb, wps, bo_sb[:, ot:ot + 1].to_broadcast((P, QBLK)),
                    mybir.AluOpType.add)
                nc.sync.dma_start(y_r[ot][:, 3 * QBLK:4 * QBLK], ysb)
            else:
                run_units(wo_units(3, ot))


_CACHE = {}


def _get_program():
    if "nc" not in _CACHE:
        nc = bacc.Bacc("TRN2", target_bir_lowering=False, debug=False,
                       num_devices=N_CORES)
        _emit(nc)
        nc.compile()
        _CACHE["nc"] = nc
    return _CACHE["nc"]


def _run(inputs, trace=False, **kwargs):
    import ml_dtypes
    nc = _get_program()
    bf16 = ml_dtypes.bfloat16
    x = np.ascontiguousarray(np.asarray(inputs["x"], dtype=np.float32)).astype(bf16)
    shared = {nm + "t": np.ascontiguousarray(
                  np.asarray(inputs[nm], dtype=np.float32).T).astype(bf16)
              for nm in W_NAMES}
    shared["bo"] = np.ascontiguousarray(np.asarray(inputs["bo"], dtype=np.float32))
    in_maps = [{"x": np.ascontiguousarray(x[i]), **shared} for i in range(N_CORES)]
    res = run_bass_kernel_spmd(nc, in_maps, core_ids=list(range(N_CORES)),
                               trace=trace, **kwargs)
    y = np.stack([np.asarray(res.results[i]["y"]) for i in range(N_CORES)], axis=0)
    return y, res


def kernel(x, Wq, Wk, Wv, Wo, bo):
    y, _ = _run({"x": x, "wq": Wq, "wk": Wk, "wv": Wv, "wo": Wo, "bo": bo})
    return y
